# revision 29
# baseline (speedup 1.0000x reference)
# Trainium2 Bass kernel for GPT-J-style cosine attention (no softmax).
#
# Reference computation (B=2, S=1024, E=2048, H=16, HD=128, ROT=64):
#   q/k/v = hs @ W.T ; partial rotary on first 64 dims of each head;
#   v /= max(count^sigmoid(norm_const), 1); q,k L2-normalized; q,k,v
#   masked by attention_mask==0 rows; attn = tril(q @ k.T) (zeros, no
#   softmax); out = (attn @ v) @ w_o.T.
#
# Sharding: core c = b*4 + g  (b in 0..1 batch, g in 0..3 head-group of
# 4 heads). Each core computes its batch's S x 512 slice of q/k/v, runs
# attention for its 4 heads, and produces a partial [S, E] out-proj
# contribution; the host sums the 4 partials per batch.
#
# All matmul operands are bf16 (halves DMA + faster PE); accumulation
# stays fp32 in PSUM. The schedule is built to keep the PE continuously
# busy (idle gaps reset the HW clock p-state to 1.2/0.65 GHz):
#  - warmup/filler matmuls on a memset tile cover the DMA fill window
#  - Q projection runs k-outer over 6 PSUM banks so per-k PE work
#    exceeds the DMA arrival pace of the (hs_k, wq_k) tile stream
#  - transposes of q/k lag into the following projection phases
#  - attention evictions are spread over Vector/GpSimd/Scalar engines
#    and the all-zero column ranges of diagonal blocks are skipped.
import numpy as np

B, S, E, H, HD, ROT, MAXP = 2, 1024, 2048, 16, 128, 64, 2048
HL = 4            # heads per core
GD = HL * HD      # 512 output dims per core
NB = S // 128     # 8 s-blocks
NK = E // 128     # 16 contraction tiles
EPS = 1e-12
WARM = 10         # warmup matmuls before first projection matmul
KOUT = 7          # m-blocks accumulated k-outer during the DMA fill
FILL = 16         # filler matmuls between k-outer end and m7
FILL_EARLY = {0: 4, 1: 8, 2: 4}  # fillers inside the k-outer loop
VEC_SS = True     # sum-of-squares on Vector (tensor_tensor_reduce) vs Scalar
SCL_QN = True     # qn eviction on Scalar (activation w/ scale) vs Vector


def _sinusoidal(num_pos, dim):
    inv_freq = 1.0 / (10000.0 ** (np.arange(0, dim, 2, dtype=np.float32) / dim))
    sinusoid = np.einsum("i,j->ij", np.arange(num_pos, dtype=np.float32), inv_freq)
    return np.concatenate([np.sin(sinusoid), np.cos(sinusoid)], axis=-1)


_BUILT = None


def _build():
    global _BUILT
    if _BUILT is not None:
        return _BUILT
    import concourse.bacc as bacc
    import concourse.mybir as mybir
    import concourse.bass as bass
    from concourse.tile import TileContext

    F32 = mybir.dt.float32
    BF16 = mybir.dt.bfloat16
    MUL = mybir.AluOpType.mult
    SQUARE = mybir.ActivationFunctionType.Square

    nc = bacc.Bacc(None, target_bir_lowering=False)

    hsd = nc.dram_tensor("hsp", [128, NK, S], BF16, kind="ExternalInput")
    wqd = nc.dram_tensor("wqp", [128, NK, GD], BF16, kind="ExternalInput")
    wkd = nc.dram_tensor("wkp", [128, NK, GD], BF16, kind="ExternalInput")
    wvd = nc.dram_tensor("wvp", [128, NK, GD], BF16, kind="ExternalInput")
    wod = nc.dram_tensor("wop", [128, 4, HL, 512], BF16, kind="ExternalInput")
    cos4d = nc.dram_tensor("cos4", [128, NB, HL, ROT], BF16, kind="ExternalInput")
    sin4d = nc.dram_tensor("sin4", [128, NB, HL, ROT], BF16, kind="ExternalInput")
    masksd = nc.dram_tensor("masks", [128, 4, 512], F32, kind="ExternalInput")
    vscaled = nc.dram_tensor("vscale", [128, NB, HL], F32, kind="ExternalInput")
    qmaskd = nc.dram_tensor("qmask", [128, NB], F32, kind="ExternalInput")
    identd = nc.dram_tensor("ident", [128, 128], BF16, kind="ExternalInput")
    outd = nc.dram_tensor("out", [S, E], BF16, kind="ExternalOutput")

    with TileContext(nc) as tc:
        from contextlib import ExitStack
        ctx = ExitStack()
        with ctx:
            const = ctx.enter_context(tc.tile_pool(name="const", bufs=1))
            data = ctx.enter_context(tc.tile_pool(name="data", bufs=1))
            scr = ctx.enter_context(tc.tile_pool(name="scr", bufs=4))
            rot_pool = ctx.enter_context(tc.tile_pool(name="rot", bufs=2))

            junk = const.tile([128, 512], BF16)
            cos4 = const.tile([128, NB, HL, ROT], BF16)
            sin4 = const.tile([128, NB, HL, ROT], BF16)
            masks = const.tile([128, 4, 512], F32)
            vscale = const.tile([128, NB, HL], F32)
            qmask = const.tile([128, NB], F32)
            ident = const.tile([128, 128], BF16)

            # junk tile for warmup matmuls: locally initialized, no DMA wait
            nc.gpsimd.memset(junk[:], 0.125)
            # small consts on the scalar queue (land within ~10us)

            # persistent data tiles
            hs = data.tile([128, NK * S], BF16)
            wq = data.tile([128, NK, GD], BF16)
            wk = data.tile([128, NK, GD], BF16)
            wv = data.tile([128, NK, GD], BF16)
            qT = [data.tile([128, S], BF16, name=f"qT{h}") for h in range(HL)]
            kT = [data.tile([128, S], BF16, name=f"kT{h}") for h in range(HL)]
            vn = [data.tile([128, GD], BF16, name=f"vn{m}") for m in range(NB)]
            aT = [data.tile([128, S], BF16, name=f"aT{h}") for h in range(HL)]
            wo = data.tile([128, 4, HL, 512], BF16)

            # sync-queue DMA stream: fine-grained (hs_k, wq_k) pairs first
            # (paces the Q projection), then wk/wv groups, then the
            # late-needed tables. The DGE processes these in order, so the
            # early stream never competes for HBM with the late loads.
            for k in range(NK):
                q = nc.scalar if k < 2 else nc.sync
                q.dma_start(
                    out=hs[:, k * S:(k + 1) * S],
                    in_=bass.AP(hsd, k * S, [[NK * S, 128], [1, S]]))
                q.dma_start(
                    out=wq[:, k],
                    in_=bass.AP(wqd, k * GD, [[NK * GD, 128], [1, GD]]))
            nc.scalar.dma_start(out=qmask[:], in_=qmaskd[:])
            nc.scalar.dma_start(out=ident[:], in_=identd[:])
            nc.sync.dma_start(out=cos4[:], in_=cos4d[:])
            nc.sync.dma_start(out=sin4[:], in_=sin4d[:])
            for g in range(4):
                nc.sync.dma_start(
                    out=wk[:, 4 * g:4 * (g + 1)],
                    in_=bass.AP(wkd, 4 * g * GD, [[NK * GD, 128], [1, 4 * GD]]))
            for g in range(4):
                nc.sync.dma_start(
                    out=wv[:, 4 * g:4 * (g + 1)],
                    in_=bass.AP(wvd, 4 * g * GD, [[NK * GD, 128], [1, 4 * GD]]))
            nc.sync.dma_start(out=vscale[:], in_=vscaled[:])
            nc.sync.dma_start(out=masks[:], in_=masksd[:])
            nc.sync.dma_start(out=wo[:], in_=wod[:])

            ADD = mybir.AluOpType.add

            def qk_postproc(ps, m):
                # per-head L2 norms straight from PSUM (rotary is
                # norm-preserving so norms can be taken pre-rotary).
                # Split across Vector (sum-of-squares) and Scalar (scaled
                # eviction) so PSUM banks release at ~1.8us/block pace.
                ss = scr.tile([128, HL], F32, tag="ss")
                if VEC_SS:
                    # one Scalar square over all 4 heads + one Vector
                    # segmented reduce: cheapest PSUM-release path
                    sqs = scr.tile([128, HL, 128], F32, tag="sqs", bufs=2)
                    nc.scalar.activation(out=sqs[:], in_=ps[:], func=SQUARE)
                    nc.vector.tensor_reduce(out=ss[:], in_=sqs[:],
                                            axis=mybir.AxisListType.X, op=ADD)
                else:
                    sqs = scr.tile([128, 128], F32, tag="sqs", bufs=2)
                    for h in range(HL):
                        nc.scalar.activation(out=sqs[:],
                                             in_=ps[:, h * 128:(h + 1) * 128],
                                             func=SQUARE,
                                             accum_out=ss[:, h:h + 1])
                nrm = scr.tile([128, HL], F32, tag="nrm")
                nc.scalar.sqrt(nrm[:], ss[:])
                nc.vector.tensor_scalar_max(nrm[:], nrm[:], EPS)
                rr = scr.tile([128, HL], F32, tag="rr")
                nc.vector.reciprocal(rr[:], nrm[:])
                nc.vector.tensor_scalar_mul(rr[:], rr[:], qmask[:, m:m + 1])
                # PSUM -> SBUF bf16 with the per-row scale folded in
                qn = rot_pool.tile([128, HL, 128], BF16, tag="qn", bufs=10)
                for h in range(HL):
                    if h < 2:
                        nc.scalar.mul(qn[:, h], ps[:, h * 128:(h + 1) * 128],
                                      rr[:, h:h + 1])
                    else:
                        nc.vector.tensor_scalar_mul(qn[:, h],
                                                    ps[:, h * 128:(h + 1) * 128],
                                                    rr[:, h:h + 1])
                # GPT-J interleaved rotary on first ROT dims of each head
                qrot = rot_pool.tile([128, HL, ROT], BF16, tag="qrot", bufs=2)
                tmp2 = rot_pool.tile([128, HL, ROT], BF16, tag="tmp2", bufs=2)
                nc.gpsimd.tensor_tensor(out=qrot[:, :, 0:ROT:2], in0=qn[:, :, 1:ROT:2],
                                        in1=sin4[:, m, :, 0:ROT:2], op=MUL)
                nc.gpsimd.tensor_tensor(out=qrot[:, :, 1:ROT:2], in0=qn[:, :, 0:ROT:2],
                                        in1=sin4[:, m, :, 1:ROT:2], op=MUL)
                nc.gpsimd.tensor_tensor(out=tmp2[:], in0=qn[:, :, 0:ROT],
                                        in1=cos4[:, m], op=MUL)
                nc.gpsimd.tensor_add(out=qn[:, :, 0:ROT], in0=qrot[:], in1=tmp2[:])
                return qn

            # One PSUM pool for the whole kernel (no pool-transition
            # barriers): ps(3) + pt(1) + pa(2) + po(2) = 8 banks.
            with tc.tile_pool(name="psB", bufs=1, space="PSUM") as psB, \
                 tc.tile_pool(name="atn", bufs=8) as atn_pool, \
                 tc.tile_pool(name="ost", bufs=3) as ost_pool:

                def fillers(n):
                    for _ in range(n):
                        wp = psB.tile([128, 512], F32, tag="pt", bufs=1, name="wp")
                        nc.tensor.matmul(wp[:], junk[:, 0:128], junk[:],
                                         start=True, stop=True)

                fillers(WARM)

                # ---- Q projection: k-outer over 7 m-blocks (borrowing the
                # attention pa/po banks) so the PE keeps pace with the
                # arriving (hs_k, wq_k) DMA stream.
                ptags = [("ps", 3)] * 3 + [("pa", 2)] * 2 + [("po", 2)] * 2
                pss = [psB.tile([128, GD], F32, tag=t, bufs=b, name=f"pss{i}")
                       for i, (t, b) in enumerate(ptags[:KOUT])]
                for k in range(NK):
                    for mi in range(KOUT):
                        nc.tensor.matmul(
                            pss[mi][:], hs[:, k * S + mi * 128: k * S + (mi + 1) * 128],
                            wq[:, k], start=(k == 0), stop=(k == NK - 1))
                    fillers(FILL_EARLY.get(k, 0))
                qns = {}
                for mi in range(KOUT):
                    qns[mi] = qk_postproc(pss[mi], mi)
                fillers(FILL)  # cover the PSUM-bank recycle latency

                def proj_block(wt, m):
                    ps = psB.tile([128, GD], F32, tag="ps", bufs=3, name="psb")
                    for k in range(NK):
                        nc.tensor.matmul(
                            ps[:], hs[:, k * S + m * 128: k * S + (m + 1) * 128],
                            wt[:, k], start=(k == 0), stop=(k == NK - 1))
                    return ps

                for m in range(KOUT, NB):
                    qns[m] = qk_postproc(proj_block(wq, m), m)

                def transpose_block(qn, m, dstT):
                    for h in range(HL):
                        pt = psB.tile([128, 128], BF16, tag="pt", bufs=1, name="pt")
                        nc.tensor.transpose(pt[:], qn[:, h], ident[:])
                        nc.vector.tensor_copy(dstT[h][:, m * 128:(m + 1) * 128], pt[:])

                def attn_unit(h, c):
                    nblk = 4 * (c + 1)
                    po = psB.tile([128, 512], F32, tag="po", bufs=2, name="po")
                    ats = []

                    def qk(j):
                        jj = j - 4 * c
                        off = max(jj, 0) * 128  # all-zero cols skipped
                        pa = psB.tile([128, 512], F32, tag="pa", bufs=2, name="pa")
                        nc.tensor.matmul(pa[:, off:512],
                                         kT[h][:, j * 128:(j + 1) * 128],
                                         qT[h][:, c * 512 + off:(c + 1) * 512],
                                         start=True, stop=True)
                        at = atn_pool.tile([128, 512], BF16, tag="at")
                        if jj >= 0:  # diagonal band: apply causal mask
                            nc.vector.tensor_tensor(out=at[:, off:512],
                                                    in0=pa[:, off:512],
                                                    in1=masks[:, jj, off:512],
                                                    op=MUL)
                        else:        # fully below the diagonal
                            nc.scalar.copy(at[:], pa[:])
                        ats.append((at, off))

                    def av(j):
                        at, off = ats[j]
                        nc.tensor.matmul(po[:, off:512],
                                         vn[j][:, h * 128:(h + 1) * 128],
                                         at[:, off:512],
                                         start=(j == 0), stop=(j == nblk - 1))

                    for j in range(nblk):
                        qk(j)
                        if j >= 1:
                            av(j - 1)
                    av(nblk - 1)
                    nc.scalar.copy(aT[h][:, c * 512:(c + 1) * 512], po[:])

                def outproj_block(m):
                    ot = ost_pool.tile([128, E], BF16, tag="ot")
                    for n in range(4):
                        ps = psB.tile([128, 512], F32, tag="ps", bufs=3, name="pso")
                        for kk in range(HL):
                            nc.tensor.matmul(ps[:], aT[kk][:, m * 128:(m + 1) * 128],
                                             wo[:, n, kk],
                                             start=(kk == 0), stop=(kk == HL - 1))
                        dst = ot[:, n * 512:(n + 1) * 512]
                        if n % 2 == 0:
                            nc.vector.tensor_copy(dst, ps[:])
                        else:
                            nc.scalar.copy(dst, ps[:])
                        # per-chunk DMA right after the evict: shortens the
                        # final drain after the last matmul
                        nc.sync.dma_start(
                            out=outd[m * 128:(m + 1) * 128, n * 512:(n + 1) * 512],
                            in_=dst)

                # K projection; interleave lagged Q transposes
                kns = {}
                for m in range(NB):
                    ps = proj_block(wk, m)
                    kns[m] = qk_postproc(ps, m)
                    transpose_block(qns.pop(m), m, qT)

                # V projection; interleave lagged K transposes, then the
                # attention c=0 units (Vector-eviction-heavy) into the
                # PE-rich projection stream
                for m in range(NB):
                    ps = proj_block(wv, m)
                    for h in range(HL):
                        nc.vector.tensor_scalar_mul(vn[m][:, h * 128:(h + 1) * 128],
                                                    ps[:, h * 128:(h + 1) * 128],
                                                    vscale[:, m, h:h + 1])
                    transpose_block(kns.pop(m), m, kT)
                    if m >= 4:
                        attn_unit(m - 4, 0)

                # attention c=1 units interleaved with out-proj blocks
                for i in range(4):
                    attn_unit(i, 1)
                    outproj_block(i)
                for m in range(4, NB):
                    outproj_block(m)

    nc.compile()
    _BUILT = nc
    return nc


def _prep_inputs(hidden_states, w_q, w_k, w_v, w_o, norm_const,
                 attention_mask, position_ids):
    """Host-side shard + table prep. Returns list of 8 in_maps."""
    import ml_dtypes
    BF = ml_dtypes.bfloat16
    hidden_states = np.asarray(hidden_states, dtype=np.float32)
    w_q = np.asarray(w_q, dtype=np.float32)
    w_k = np.asarray(w_k, dtype=np.float32)
    w_v = np.asarray(w_v, dtype=np.float32)
    w_o = np.asarray(w_o, dtype=np.float32)
    norm_const = np.asarray(norm_const, dtype=np.float32).reshape(H)
    attention_mask = np.asarray(attention_mask, dtype=np.float32).reshape(B, S)
    position_ids = np.asarray(position_ids).reshape(B, S).astype(np.int64)

    embed = _sinusoidal(MAXP, ROT)                       # [MAXP, 64]
    sig = 1.0 / (1.0 + np.exp(-norm_const.astype(np.float64)))   # [H]
    mask0 = (attention_mask == 0).astype(np.float32)     # [B, S]
    counts = np.cumsum(mask0, axis=1).astype(np.float32)  # [B, S]
    denom = np.maximum(counts[:, None, :] ** sig[None, :, None], 1.0).astype(np.float32)
    vs_full = mask0[:, None, :] / denom                  # [B, H, S]

    # causal masks for the 4 diagonal-band block offsets
    p = np.arange(128)[:, None]
    f = np.arange(512)[None, :]
    masks = np.stack([(jj * 128 + p <= f) for jj in range(4)]).astype(np.float32)
    masks = np.ascontiguousarray(masks.transpose(1, 0, 2))  # [128, 4, 512]
    ident = np.eye(128, dtype=np.float32).astype(BF)

    def shuffle_k(a):  # [NK*128, F] -> [128, NK, F] (partition-major)
        nk, f = a.shape[0] // 128, a.shape[1]
        return np.ascontiguousarray(
            a.reshape(nk, 128, f).transpose(1, 0, 2).astype(BF))

    in_maps = []
    for b in range(B):
        sincos = embed[position_ids[b]]                  # [S, 64]
        sin, cos = sincos[:, :ROT // 2], sincos[:, ROT // 2:]
        cosR = np.repeat(cos, 2, axis=1)                 # [S, 64]
        sinS = np.empty((S, ROT), dtype=np.float32)
        sinS[:, 0::2] = -sin
        sinS[:, 1::2] = sin
        # [S,64] -> [128 part, NB, 64] -> broadcast over HL heads
        def to4(t):
            t = t.reshape(NB, 128, ROT).transpose(1, 0, 2)
            return np.ascontiguousarray(
                np.broadcast_to(t[:, :, None, :], (128, NB, HL, ROT))).astype(BF)
        cos4 = to4(cosR)
        sin4 = to4(sinS)
        qm = np.ascontiguousarray(mask0[b].reshape(NB, 128).T)  # [128, NB]
        hsp_b = shuffle_k(np.ascontiguousarray(hidden_states[b].T))  # [128,NK,S]
        for g in range(4):
            sl = slice(g * GD, (g + 1) * GD)
            vs = vs_full[b, 4 * g:4 * g + HL, :]                # [HL, S]
            vs = np.ascontiguousarray(
                vs.reshape(HL, NB, 128).transpose(2, 1, 0))     # [128, NB, HL]
            # w_o[:, sl] is [E, GD]; wop[p, n, kk, c] = w_o[n*512+c, kk*128+p]
            wop = np.ascontiguousarray(
                w_o[:, sl].reshape(4, 512, HL, 128).transpose(3, 0, 2, 1)).astype(BF)
            in_maps.append({
                "hsp": hsp_b,
                "wqp": shuffle_k(np.ascontiguousarray(w_q[sl, :].T)),
                "wkp": shuffle_k(np.ascontiguousarray(w_k[sl, :].T)),
                "wvp": shuffle_k(np.ascontiguousarray(w_v[sl, :].T)),
                "wop": wop,
                "cos4": cos4, "sin4": sin4, "masks": masks,
                "vscale": vs, "qmask": qm, "ident": ident,
            })
    # core order: c = b*4 + g
    return in_maps


def run(inputs, trace=False, trace_cores=None):
    from concourse.bass_utils import run_bass_kernel_spmd
    nc = _build()
    in_maps = _prep_inputs(**inputs)
    res = run_bass_kernel_spmd(nc, in_maps, core_ids=list(range(8)),
                               trace=trace, trace_cores=trace_cores)
    out = np.empty((B, S, E), dtype=np.float32)
    for b in range(B):
        acc = np.zeros((S, E), dtype=np.float32)
        for g in range(4):
            acc += np.asarray(res.results[4 * b + g]["out"], dtype=np.float32)
        out[b] = acc
    return out, res


def kernel(**inputs):
    out, _ = run(inputs, trace=False)
    return out


# revision 30
# speedup vs baseline: 1.0150x; 1.0150x over previous
# Trainium2 Bass kernel for GPT-J-style cosine attention (no softmax).
#
# Reference computation (B=2, S=1024, E=2048, H=16, HD=128, ROT=64):
#   q/k/v = hs @ W.T ; partial rotary on first 64 dims of each head;
#   v /= max(count^sigmoid(norm_const), 1); q,k L2-normalized; q,k,v
#   masked by attention_mask==0 rows; attn = tril(q @ k.T) (zeros, no
#   softmax); out = (attn @ v) @ w_o.T.
#
# Sharding: core c = b*4 + g  (b in 0..1 batch, g in 0..3 head-group of
# 4 heads). Each core computes its batch's S x 512 slice of q/k/v, runs
# attention for its 4 heads, and produces a partial [S, E] out-proj
# contribution; the host sums the 4 partials per batch.
#
# All matmul operands are bf16 (halves DMA + faster PE); accumulation
# stays fp32 in PSUM. The schedule is built to keep the PE continuously
# busy (idle gaps reset the HW clock p-state to 1.2/0.65 GHz):
#  - warmup/filler matmuls on a memset tile cover the DMA fill window
#  - Q projection runs k-outer over 6 PSUM banks so per-k PE work
#    exceeds the DMA arrival pace of the (hs_k, wq_k) tile stream
#  - transposes of q/k lag into the following projection phases
#  - attention evictions are spread over Vector/GpSimd/Scalar engines
#    and the all-zero column ranges of diagonal blocks are skipped.
import numpy as np

B, S, E, H, HD, ROT, MAXP = 2, 1024, 2048, 16, 128, 64, 2048
HL = 4            # heads per core
GD = HL * HD      # 512 output dims per core
NB = S // 128     # 8 s-blocks
NK = E // 128     # 16 contraction tiles
EPS = 1e-12
WARM = 10         # warmup matmuls before first projection matmul
KOUT = 7          # m-blocks accumulated k-outer during the DMA fill
FILL = 16         # filler matmuls between k-outer end and m7
FILL_EARLY = {0: 4, 1: 8, 2: 4}  # fillers inside the k-outer loop
VEC_SS = True     # sum-of-squares on Vector (tensor_tensor_reduce) vs Scalar
SCL_QN = True     # qn eviction on Scalar (activation w/ scale) vs Vector


def _sinusoidal(num_pos, dim):
    inv_freq = 1.0 / (10000.0 ** (np.arange(0, dim, 2, dtype=np.float32) / dim))
    sinusoid = np.einsum("i,j->ij", np.arange(num_pos, dtype=np.float32), inv_freq)
    return np.concatenate([np.sin(sinusoid), np.cos(sinusoid)], axis=-1)


_BUILT = None


def _build():
    global _BUILT
    if _BUILT is not None:
        return _BUILT
    import concourse.bacc as bacc
    import concourse.mybir as mybir
    import concourse.bass as bass
    from concourse.tile import TileContext

    F32 = mybir.dt.float32
    BF16 = mybir.dt.bfloat16
    MUL = mybir.AluOpType.mult
    SQUARE = mybir.ActivationFunctionType.Square

    nc = bacc.Bacc(None, target_bir_lowering=False)

    hsd = nc.dram_tensor("hsp", [128, NK, S], BF16, kind="ExternalInput")
    wqd = nc.dram_tensor("wqp", [128, NK, GD], BF16, kind="ExternalInput")
    wkd = nc.dram_tensor("wkp", [128, NK, GD], BF16, kind="ExternalInput")
    wvd = nc.dram_tensor("wvp", [128, NK, GD], BF16, kind="ExternalInput")
    wod = nc.dram_tensor("wop", [128, 4, HL, 512], BF16, kind="ExternalInput")
    cos4d = nc.dram_tensor("cos4", [128, NB, HL, ROT], BF16, kind="ExternalInput")
    sin4d = nc.dram_tensor("sin4", [128, NB, HL, ROT], BF16, kind="ExternalInput")
    masksd = nc.dram_tensor("masks", [128, 4, 512], F32, kind="ExternalInput")
    vscaled = nc.dram_tensor("vscale", [128, NB, HL], F32, kind="ExternalInput")
    qmaskd = nc.dram_tensor("qmask", [128, NB], F32, kind="ExternalInput")
    identd = nc.dram_tensor("ident", [128, 128], BF16, kind="ExternalInput")
    outd = nc.dram_tensor("out", [S, E], BF16, kind="ExternalOutput")

    with TileContext(nc) as tc:
        from contextlib import ExitStack
        ctx = ExitStack()
        with ctx:
            const = ctx.enter_context(tc.tile_pool(name="const", bufs=1))
            data = ctx.enter_context(tc.tile_pool(name="data", bufs=1))
            scr = ctx.enter_context(tc.tile_pool(name="scr", bufs=4))
            rot_pool = ctx.enter_context(tc.tile_pool(name="rot", bufs=2))

            junk = const.tile([128, 512], BF16)
            cos4 = const.tile([128, NB, HL, ROT], BF16)
            sin4 = const.tile([128, NB, HL, ROT], BF16)
            masks = const.tile([128, 4, 512], F32)
            vscale = const.tile([128, NB, HL], F32)
            qmask = const.tile([128, NB], F32)
            ident = const.tile([128, 128], BF16)

            # junk tile for warmup matmuls: locally initialized, no DMA wait
            nc.gpsimd.memset(junk[:], 0.125)
            # small consts on the scalar queue (land within ~10us)

            # persistent data tiles
            hs = data.tile([128, NK * S], BF16)
            wq = data.tile([128, NK, GD], BF16)
            wk = data.tile([128, NK, GD], BF16)
            wv = data.tile([128, NK, GD], BF16)
            qT = [data.tile([128, S], BF16, name=f"qT{h}") for h in range(HL)]
            kT = [data.tile([128, S], BF16, name=f"kT{h}") for h in range(HL)]
            vn = [data.tile([128, GD], BF16, name=f"vn{m}") for m in range(NB)]
            aT = [data.tile([128, S], BF16, name=f"aT{h}") for h in range(HL)]
            wo = data.tile([128, 4, HL, 512], BF16)

            # sync-queue DMA stream: fine-grained (hs_k, wq_k) pairs first
            # (paces the Q projection), then wk/wv groups, then the
            # late-needed tables. The DGE processes these in order, so the
            # early stream never competes for HBM with the late loads.
            nc.scalar.dma_start(out=qmask[:], in_=qmaskd[:])
            nc.scalar.dma_start(out=ident[:], in_=identd[:])
            for k in range(NK):
                nc.sync.dma_start(
                    out=hs[:, k * S:(k + 1) * S],
                    in_=bass.AP(hsd, k * S, [[NK * S, 128], [1, S]]))
                nc.sync.dma_start(
                    out=wq[:, k],
                    in_=bass.AP(wqd, k * GD, [[NK * GD, 128], [1, GD]]))
            nc.sync.dma_start(out=cos4[:], in_=cos4d[:])
            nc.sync.dma_start(out=sin4[:], in_=sin4d[:])
            for g in range(4):
                nc.sync.dma_start(
                    out=wk[:, 4 * g:4 * (g + 1)],
                    in_=bass.AP(wkd, 4 * g * GD, [[NK * GD, 128], [1, 4 * GD]]))
            for g in range(4):
                nc.sync.dma_start(
                    out=wv[:, 4 * g:4 * (g + 1)],
                    in_=bass.AP(wvd, 4 * g * GD, [[NK * GD, 128], [1, 4 * GD]]))
            nc.sync.dma_start(out=vscale[:], in_=vscaled[:])
            nc.sync.dma_start(out=masks[:], in_=masksd[:])
            nc.sync.dma_start(out=wo[:], in_=wod[:])

            ADD = mybir.AluOpType.add

            def qk_postproc(ps, m):
                # per-head L2 norms straight from PSUM (rotary is
                # norm-preserving so norms can be taken pre-rotary).
                # Split across Vector (sum-of-squares) and Scalar (scaled
                # eviction) so PSUM banks release at ~1.8us/block pace.
                ss = scr.tile([128, HL], F32, tag="ss")
                if VEC_SS:
                    # one Scalar square over all 4 heads + one Vector
                    # segmented reduce: cheapest PSUM-release path
                    sqs = scr.tile([128, HL, 128], F32, tag="sqs", bufs=2)
                    nc.scalar.activation(out=sqs[:], in_=ps[:], func=SQUARE)
                    nc.vector.tensor_reduce(out=ss[:], in_=sqs[:],
                                            axis=mybir.AxisListType.X, op=ADD)
                else:
                    sqs = scr.tile([128, 128], F32, tag="sqs", bufs=2)
                    for h in range(HL):
                        nc.scalar.activation(out=sqs[:],
                                             in_=ps[:, h * 128:(h + 1) * 128],
                                             func=SQUARE,
                                             accum_out=ss[:, h:h + 1])
                nrm = scr.tile([128, HL], F32, tag="nrm")
                nc.scalar.sqrt(nrm[:], ss[:])
                nc.vector.tensor_scalar_max(nrm[:], nrm[:], EPS)
                rr = scr.tile([128, HL], F32, tag="rr")
                nc.vector.reciprocal(rr[:], nrm[:])
                nc.vector.tensor_scalar_mul(rr[:], rr[:], qmask[:, m:m + 1])
                # PSUM -> SBUF bf16 with the per-row scale folded in
                qn = rot_pool.tile([128, HL, 128], BF16, tag="qn", bufs=10)
                for h in range(HL):
                    nc.scalar.mul(qn[:, h], ps[:, h * 128:(h + 1) * 128],
                                  rr[:, h:h + 1])
                # GPT-J interleaved rotary on first ROT dims of each head
                qrot = rot_pool.tile([128, HL, ROT], BF16, tag="qrot", bufs=2)
                tmp2 = rot_pool.tile([128, HL, ROT], BF16, tag="tmp2", bufs=2)
                nc.gpsimd.tensor_tensor(out=qrot[:, :, 0:ROT:2], in0=qn[:, :, 1:ROT:2],
                                        in1=sin4[:, m, :, 0:ROT:2], op=MUL)
                nc.gpsimd.tensor_tensor(out=qrot[:, :, 1:ROT:2], in0=qn[:, :, 0:ROT:2],
                                        in1=sin4[:, m, :, 1:ROT:2], op=MUL)
                nc.gpsimd.tensor_tensor(out=tmp2[:], in0=qn[:, :, 0:ROT],
                                        in1=cos4[:, m], op=MUL)
                nc.gpsimd.tensor_add(out=qn[:, :, 0:ROT], in0=qrot[:], in1=tmp2[:])
                return qn

            # One PSUM pool for the whole kernel (no pool-transition
            # barriers): ps(3) + pt(1) + pa(2) + po(2) = 8 banks.
            with tc.tile_pool(name="psB", bufs=1, space="PSUM") as psB, \
                 tc.tile_pool(name="atn", bufs=8) as atn_pool, \
                 tc.tile_pool(name="ost", bufs=3) as ost_pool:

                def fillers(n):
                    for _ in range(n):
                        wp = psB.tile([128, 512], F32, tag="pt", bufs=1, name="wp")
                        nc.tensor.matmul(wp[:], junk[:, 0:128], junk[:],
                                         start=True, stop=True)

                fillers(WARM)

                # ---- Q projection: k-outer over 7 m-blocks (borrowing the
                # attention pa/po banks) so the PE keeps pace with the
                # arriving (hs_k, wq_k) DMA stream.
                ptags = [("ps", 3)] * 3 + [("pa", 2)] * 2 + [("po", 2)] * 2
                pss = [psB.tile([128, GD], F32, tag=t, bufs=b, name=f"pss{i}")
                       for i, (t, b) in enumerate(ptags[:KOUT])]
                for k in range(NK):
                    for mi in range(KOUT):
                        nc.tensor.matmul(
                            pss[mi][:], hs[:, k * S + mi * 128: k * S + (mi + 1) * 128],
                            wq[:, k], start=(k == 0), stop=(k == NK - 1))
                    fillers(FILL_EARLY.get(k, 0))
                qns = {}
                for mi in range(KOUT):
                    qns[mi] = qk_postproc(pss[mi], mi)
                fillers(FILL)  # cover the PSUM-bank recycle latency

                def proj_block(wt, m):
                    ps = psB.tile([128, GD], F32, tag="ps", bufs=3, name="psb")
                    for k in range(NK):
                        nc.tensor.matmul(
                            ps[:], hs[:, k * S + m * 128: k * S + (m + 1) * 128],
                            wt[:, k], start=(k == 0), stop=(k == NK - 1))
                    return ps

                for m in range(KOUT, NB):
                    qns[m] = qk_postproc(proj_block(wq, m), m)

                def transpose_block(qn, m, dstT):
                    for h in range(HL):
                        pt = psB.tile([128, 128], BF16, tag="pt", bufs=1, name="pt")
                        nc.tensor.transpose(pt[:], qn[:, h], ident[:])
                        nc.vector.tensor_copy(dstT[h][:, m * 128:(m + 1) * 128], pt[:])

                def attn_unit(h, c):
                    nblk = 4 * (c + 1)
                    po = psB.tile([128, 512], F32, tag="po", bufs=2, name="po")
                    ats = []

                    def qk(j):
                        jj = j - 4 * c
                        off = max(jj, 0) * 128  # all-zero cols skipped
                        pa = psB.tile([128, 512], F32, tag="pa", bufs=2, name="pa")
                        nc.tensor.matmul(pa[:, off:512],
                                         kT[h][:, j * 128:(j + 1) * 128],
                                         qT[h][:, c * 512 + off:(c + 1) * 512],
                                         start=True, stop=True)
                        at = atn_pool.tile([128, 512], BF16, tag="at")
                        if jj >= 0:  # diagonal band: apply causal mask
                            nc.vector.tensor_tensor(out=at[:, off:512],
                                                    in0=pa[:, off:512],
                                                    in1=masks[:, jj, off:512],
                                                    op=MUL)
                        else:        # fully below the diagonal
                            nc.scalar.copy(at[:], pa[:])
                        ats.append((at, off))

                    def av(j):
                        at, off = ats[j]
                        nc.tensor.matmul(po[:, off:512],
                                         vn[j][:, h * 128:(h + 1) * 128],
                                         at[:, off:512],
                                         start=(j == 0), stop=(j == nblk - 1))

                    for j in range(nblk):
                        qk(j)
                        if j >= 1:
                            av(j - 1)
                    av(nblk - 1)
                    nc.scalar.copy(aT[h][:, c * 512:(c + 1) * 512], po[:])

                def outproj_block(m):
                    ot = ost_pool.tile([128, E], BF16, tag="ot")
                    for n in range(4):
                        ps = psB.tile([128, 512], F32, tag="ps", bufs=3, name="pso")
                        for kk in range(HL):
                            nc.tensor.matmul(ps[:], aT[kk][:, m * 128:(m + 1) * 128],
                                             wo[:, n, kk],
                                             start=(kk == 0), stop=(kk == HL - 1))
                        dst = ot[:, n * 512:(n + 1) * 512]
                        if n % 2 == 0:
                            nc.vector.tensor_copy(dst, ps[:])
                        else:
                            nc.scalar.copy(dst, ps[:])
                        # per-chunk DMA right after the evict: shortens the
                        # final drain after the last matmul
                        nc.sync.dma_start(
                            out=outd[m * 128:(m + 1) * 128, n * 512:(n + 1) * 512],
                            in_=dst)

                # K projection; interleave lagged Q transposes
                kns = {}
                for m in range(NB):
                    ps = proj_block(wk, m)
                    kns[m] = qk_postproc(ps, m)
                    transpose_block(qns.pop(m), m, qT)

                # V projection; interleave lagged K transposes, then the
                # attention c=0 units (Vector-eviction-heavy) into the
                # PE-rich projection stream
                for m in range(NB):
                    ps = proj_block(wv, m)
                    for h in range(HL):
                        nc.vector.tensor_scalar_mul(vn[m][:, h * 128:(h + 1) * 128],
                                                    ps[:, h * 128:(h + 1) * 128],
                                                    vscale[:, m, h:h + 1])
                    transpose_block(kns.pop(m), m, kT)
                    if m >= 4:
                        attn_unit(m - 4, 0)

                # attention c=1 units interleaved with out-proj blocks
                for i in range(4):
                    attn_unit(i, 1)
                    outproj_block(i)
                for m in range(4, NB):
                    outproj_block(m)

    nc.compile()
    _BUILT = nc
    return nc


def _prep_inputs(hidden_states, w_q, w_k, w_v, w_o, norm_const,
                 attention_mask, position_ids):
    """Host-side shard + table prep. Returns list of 8 in_maps."""
    import ml_dtypes
    BF = ml_dtypes.bfloat16
    hidden_states = np.asarray(hidden_states, dtype=np.float32)
    w_q = np.asarray(w_q, dtype=np.float32)
    w_k = np.asarray(w_k, dtype=np.float32)
    w_v = np.asarray(w_v, dtype=np.float32)
    w_o = np.asarray(w_o, dtype=np.float32)
    norm_const = np.asarray(norm_const, dtype=np.float32).reshape(H)
    attention_mask = np.asarray(attention_mask, dtype=np.float32).reshape(B, S)
    position_ids = np.asarray(position_ids).reshape(B, S).astype(np.int64)

    embed = _sinusoidal(MAXP, ROT)                       # [MAXP, 64]
    sig = 1.0 / (1.0 + np.exp(-norm_const.astype(np.float64)))   # [H]
    mask0 = (attention_mask == 0).astype(np.float32)     # [B, S]
    counts = np.cumsum(mask0, axis=1).astype(np.float32)  # [B, S]
    denom = np.maximum(counts[:, None, :] ** sig[None, :, None], 1.0).astype(np.float32)
    vs_full = mask0[:, None, :] / denom                  # [B, H, S]

    # causal masks for the 4 diagonal-band block offsets
    p = np.arange(128)[:, None]
    f = np.arange(512)[None, :]
    masks = np.stack([(jj * 128 + p <= f) for jj in range(4)]).astype(np.float32)
    masks = np.ascontiguousarray(masks.transpose(1, 0, 2))  # [128, 4, 512]
    ident = np.eye(128, dtype=np.float32).astype(BF)

    def shuffle_k(a):  # [NK*128, F] -> [128, NK, F] (partition-major)
        nk, f = a.shape[0] // 128, a.shape[1]
        return np.ascontiguousarray(
            a.reshape(nk, 128, f).transpose(1, 0, 2).astype(BF))

    in_maps = []
    for b in range(B):
        sincos = embed[position_ids[b]]                  # [S, 64]
        sin, cos = sincos[:, :ROT // 2], sincos[:, ROT // 2:]
        cosR = np.repeat(cos, 2, axis=1)                 # [S, 64]
        sinS = np.empty((S, ROT), dtype=np.float32)
        sinS[:, 0::2] = -sin
        sinS[:, 1::2] = sin
        # [S,64] -> [128 part, NB, 64] -> broadcast over HL heads
        def to4(t):
            t = t.reshape(NB, 128, ROT).transpose(1, 0, 2)
            return np.ascontiguousarray(
                np.broadcast_to(t[:, :, None, :], (128, NB, HL, ROT))).astype(BF)
        cos4 = to4(cosR)
        sin4 = to4(sinS)
        qm = np.ascontiguousarray(mask0[b].reshape(NB, 128).T)  # [128, NB]
        hsp_b = shuffle_k(np.ascontiguousarray(hidden_states[b].T))  # [128,NK,S]
        for g in range(4):
            sl = slice(g * GD, (g + 1) * GD)
            vs = vs_full[b, 4 * g:4 * g + HL, :]                # [HL, S]
            vs = np.ascontiguousarray(
                vs.reshape(HL, NB, 128).transpose(2, 1, 0))     # [128, NB, HL]
            # w_o[:, sl] is [E, GD]; wop[p, n, kk, c] = w_o[n*512+c, kk*128+p]
            wop = np.ascontiguousarray(
                w_o[:, sl].reshape(4, 512, HL, 128).transpose(3, 0, 2, 1)).astype(BF)
            in_maps.append({
                "hsp": hsp_b,
                "wqp": shuffle_k(np.ascontiguousarray(w_q[sl, :].T)),
                "wkp": shuffle_k(np.ascontiguousarray(w_k[sl, :].T)),
                "wvp": shuffle_k(np.ascontiguousarray(w_v[sl, :].T)),
                "wop": wop,
                "cos4": cos4, "sin4": sin4, "masks": masks,
                "vscale": vs, "qmask": qm, "ident": ident,
            })
    # core order: c = b*4 + g
    return in_maps


def run(inputs, trace=False, trace_cores=None):
    from concourse.bass_utils import run_bass_kernel_spmd
    nc = _build()
    in_maps = _prep_inputs(**inputs)
    res = run_bass_kernel_spmd(nc, in_maps, core_ids=list(range(8)),
                               trace=trace, trace_cores=trace_cores)
    out = np.empty((B, S, E), dtype=np.float32)
    for b in range(B):
        acc = np.zeros((S, E), dtype=np.float32)
        for g in range(4):
            acc += np.asarray(res.results[4 * b + g]["out"], dtype=np.float32)
        out[b] = acc
    return out, res


def kernel(**inputs):
    out, _ = run(inputs, trace=False)
    return out


# revision 31
# speedup vs baseline: 1.0181x; 1.0031x over previous
# Trainium2 Bass kernel for GPT-J-style cosine attention (no softmax).
#
# Reference computation (B=2, S=1024, E=2048, H=16, HD=128, ROT=64):
#   q/k/v = hs @ W.T ; partial rotary on first 64 dims of each head;
#   v /= max(count^sigmoid(norm_const), 1); q,k L2-normalized; q,k,v
#   masked by attention_mask==0 rows; attn = tril(q @ k.T) (zeros, no
#   softmax); out = (attn @ v) @ w_o.T.
#
# Sharding: core c = b*4 + g  (b in 0..1 batch, g in 0..3 head-group of
# 4 heads). Each core computes its batch's S x 512 slice of q/k/v, runs
# attention for its 4 heads, and produces a partial [S, E] out-proj
# contribution; the host sums the 4 partials per batch.
#
# All matmul operands are bf16 (halves DMA + faster PE); accumulation
# stays fp32 in PSUM. The schedule is built to keep the PE continuously
# busy (idle gaps reset the HW clock p-state to 1.2/0.65 GHz):
#  - warmup/filler matmuls on a memset tile cover the DMA fill window
#  - Q projection runs k-outer over 6 PSUM banks so per-k PE work
#    exceeds the DMA arrival pace of the (hs_k, wq_k) tile stream
#  - transposes of q/k lag into the following projection phases
#  - attention evictions are spread over Vector/GpSimd/Scalar engines
#    and the all-zero column ranges of diagonal blocks are skipped.
import numpy as np

B, S, E, H, HD, ROT, MAXP = 2, 1024, 2048, 16, 128, 64, 2048
HL = 4            # heads per core
GD = HL * HD      # 512 output dims per core
NB = S // 128     # 8 s-blocks
NK = E // 128     # 16 contraction tiles
EPS = 1e-12
WARM = 10         # warmup matmuls before first projection matmul
KOUT = 7          # m-blocks accumulated k-outer during the DMA fill
FILL = 13         # filler matmuls between k-outer end and m7
FILL_EARLY = {0: 4, 1: 6, 2: 2}  # fillers inside the k-outer loop
VEC_SS = True     # sum-of-squares on Vector (tensor_tensor_reduce) vs Scalar
SCL_QN = True     # qn eviction on Scalar (activation w/ scale) vs Vector


def _sinusoidal(num_pos, dim):
    inv_freq = 1.0 / (10000.0 ** (np.arange(0, dim, 2, dtype=np.float32) / dim))
    sinusoid = np.einsum("i,j->ij", np.arange(num_pos, dtype=np.float32), inv_freq)
    return np.concatenate([np.sin(sinusoid), np.cos(sinusoid)], axis=-1)


_BUILT = None


def _build():
    global _BUILT
    if _BUILT is not None:
        return _BUILT
    import concourse.bacc as bacc
    import concourse.mybir as mybir
    import concourse.bass as bass
    from concourse.tile import TileContext

    F32 = mybir.dt.float32
    BF16 = mybir.dt.bfloat16
    MUL = mybir.AluOpType.mult
    SQUARE = mybir.ActivationFunctionType.Square

    nc = bacc.Bacc(None, target_bir_lowering=False)

    hsd = nc.dram_tensor("hsp", [128, NK, S], BF16, kind="ExternalInput")
    wqd = nc.dram_tensor("wqp", [128, NK, GD], BF16, kind="ExternalInput")
    wkd = nc.dram_tensor("wkp", [128, NK, GD], BF16, kind="ExternalInput")
    wvd = nc.dram_tensor("wvp", [128, NK, GD], BF16, kind="ExternalInput")
    wod = nc.dram_tensor("wop", [128, 4, HL, 512], BF16, kind="ExternalInput")
    cos4d = nc.dram_tensor("cos4", [128, NB, HL, ROT], BF16, kind="ExternalInput")
    sin4d = nc.dram_tensor("sin4", [128, NB, HL, ROT], BF16, kind="ExternalInput")
    masksd = nc.dram_tensor("masks", [128, 4, 512], F32, kind="ExternalInput")
    vscaled = nc.dram_tensor("vscale", [128, NB, HL], F32, kind="ExternalInput")
    qmaskd = nc.dram_tensor("qmask", [128, NB], F32, kind="ExternalInput")
    identd = nc.dram_tensor("ident", [128, 128], BF16, kind="ExternalInput")
    outd = nc.dram_tensor("out", [S, E], BF16, kind="ExternalOutput")

    with TileContext(nc) as tc:
        from contextlib import ExitStack
        ctx = ExitStack()
        with ctx:
            const = ctx.enter_context(tc.tile_pool(name="const", bufs=1))
            data = ctx.enter_context(tc.tile_pool(name="data", bufs=1))
            scr = ctx.enter_context(tc.tile_pool(name="scr", bufs=4))
            rot_pool = ctx.enter_context(tc.tile_pool(name="rot", bufs=2))

            junk = const.tile([128, 512], BF16)
            cos4 = const.tile([128, NB, HL, ROT], BF16)
            sin4 = const.tile([128, NB, HL, ROT], BF16)
            masks = const.tile([128, 4, 512], F32)
            vscale = const.tile([128, NB, HL], F32)
            qmask = const.tile([128, NB], F32)
            ident = const.tile([128, 128], BF16)

            # junk tile for warmup matmuls: locally initialized, no DMA wait
            nc.gpsimd.memset(junk[:], 0.125)
            # small consts on the scalar queue (land within ~10us)

            # persistent data tiles
            hs = data.tile([128, NK * S], BF16)
            wq = data.tile([128, NK, GD], BF16)
            wk = data.tile([128, NK, GD], BF16)
            wv = data.tile([128, NK, GD], BF16)
            qT = [data.tile([128, S], BF16, name=f"qT{h}") for h in range(HL)]
            kT = [data.tile([128, S], BF16, name=f"kT{h}") for h in range(HL)]
            vn = [data.tile([128, GD], BF16, name=f"vn{m}") for m in range(NB)]
            aT = [data.tile([128, S], BF16, name=f"aT{h}") for h in range(HL)]
            wo = data.tile([128, 4, HL, 512], BF16)

            # sync-queue DMA stream: fine-grained (hs_k, wq_k) pairs first
            # (paces the Q projection), then wk/wv groups, then the
            # late-needed tables. The DGE processes these in order, so the
            # early stream never competes for HBM with the late loads.
            nc.scalar.dma_start(out=qmask[:], in_=qmaskd[:])
            nc.scalar.dma_start(out=ident[:], in_=identd[:])
            for k in range(NK):
                nc.sync.dma_start(
                    out=hs[:, k * S:(k + 1) * S],
                    in_=bass.AP(hsd, k * S, [[NK * S, 128], [1, S]]))
                nc.sync.dma_start(
                    out=wq[:, k],
                    in_=bass.AP(wqd, k * GD, [[NK * GD, 128], [1, GD]]))
            nc.sync.dma_start(out=cos4[:], in_=cos4d[:])
            nc.sync.dma_start(out=sin4[:], in_=sin4d[:])
            for g in range(4):
                nc.sync.dma_start(
                    out=wk[:, 4 * g:4 * (g + 1)],
                    in_=bass.AP(wkd, 4 * g * GD, [[NK * GD, 128], [1, 4 * GD]]))
            for g in range(4):
                nc.sync.dma_start(
                    out=wv[:, 4 * g:4 * (g + 1)],
                    in_=bass.AP(wvd, 4 * g * GD, [[NK * GD, 128], [1, 4 * GD]]))
            nc.sync.dma_start(out=vscale[:], in_=vscaled[:])
            nc.sync.dma_start(out=masks[:], in_=masksd[:])
            nc.sync.dma_start(out=wo[:], in_=wod[:])

            ADD = mybir.AluOpType.add

            def qk_postproc(ps, m):
                # per-head L2 norms straight from PSUM (rotary is
                # norm-preserving so norms can be taken pre-rotary).
                # Split across Vector (sum-of-squares) and Scalar (scaled
                # eviction) so PSUM banks release at ~1.8us/block pace.
                ss = scr.tile([128, HL], F32, tag="ss")
                if VEC_SS:
                    # one Scalar square over all 4 heads + one Vector
                    # segmented reduce: cheapest PSUM-release path
                    sqs = scr.tile([128, HL, 128], F32, tag="sqs", bufs=2)
                    nc.scalar.activation(out=sqs[:], in_=ps[:], func=SQUARE)
                    nc.vector.tensor_reduce(out=ss[:], in_=sqs[:],
                                            axis=mybir.AxisListType.X, op=ADD)
                else:
                    sqs = scr.tile([128, 128], F32, tag="sqs", bufs=2)
                    for h in range(HL):
                        nc.scalar.activation(out=sqs[:],
                                             in_=ps[:, h * 128:(h + 1) * 128],
                                             func=SQUARE,
                                             accum_out=ss[:, h:h + 1])
                nrm = scr.tile([128, HL], F32, tag="nrm")
                nc.scalar.sqrt(nrm[:], ss[:])
                nc.vector.tensor_scalar_max(nrm[:], nrm[:], EPS)
                rr = scr.tile([128, HL], F32, tag="rr")
                nc.vector.reciprocal(rr[:], nrm[:])
                nc.vector.tensor_scalar_mul(rr[:], rr[:], qmask[:, m:m + 1])
                # PSUM -> SBUF bf16 with the per-row scale folded in
                qn = rot_pool.tile([128, HL, 128], BF16, tag="qn", bufs=10)
                for h in range(HL):
                    nc.scalar.mul(qn[:, h], ps[:, h * 128:(h + 1) * 128],
                                  rr[:, h:h + 1])
                # GPT-J interleaved rotary on first ROT dims of each head
                qrot = rot_pool.tile([128, HL, ROT], BF16, tag="qrot", bufs=2)
                tmp2 = rot_pool.tile([128, HL, ROT], BF16, tag="tmp2", bufs=2)
                nc.gpsimd.tensor_tensor(out=qrot[:, :, 0:ROT:2], in0=qn[:, :, 1:ROT:2],
                                        in1=sin4[:, m, :, 0:ROT:2], op=MUL)
                nc.gpsimd.tensor_tensor(out=qrot[:, :, 1:ROT:2], in0=qn[:, :, 0:ROT:2],
                                        in1=sin4[:, m, :, 1:ROT:2], op=MUL)
                nc.gpsimd.tensor_tensor(out=tmp2[:], in0=qn[:, :, 0:ROT],
                                        in1=cos4[:, m], op=MUL)
                nc.gpsimd.tensor_add(out=qn[:, :, 0:ROT], in0=qrot[:], in1=tmp2[:])
                return qn

            # One PSUM pool for the whole kernel (no pool-transition
            # barriers): ps(3) + pt(1) + pa(2) + po(2) = 8 banks.
            with tc.tile_pool(name="psB", bufs=1, space="PSUM") as psB, \
                 tc.tile_pool(name="atn", bufs=8) as atn_pool, \
                 tc.tile_pool(name="ost", bufs=3) as ost_pool:

                def fillers(n):
                    for _ in range(n):
                        wp = psB.tile([128, 512], F32, tag="pt", bufs=1, name="wp")
                        nc.tensor.matmul(wp[:], junk[:, 0:128], junk[:],
                                         start=True, stop=True)

                fillers(WARM)

                # ---- Q projection: k-outer over 7 m-blocks (borrowing the
                # attention pa/po banks) so the PE keeps pace with the
                # arriving (hs_k, wq_k) DMA stream.
                ptags = [("ps", 3)] * 3 + [("pa", 2)] * 2 + [("po", 2)] * 2
                pss = [psB.tile([128, GD], F32, tag=t, bufs=b, name=f"pss{i}")
                       for i, (t, b) in enumerate(ptags[:KOUT])]
                for k in range(NK):
                    for mi in range(KOUT):
                        nc.tensor.matmul(
                            pss[mi][:], hs[:, k * S + mi * 128: k * S + (mi + 1) * 128],
                            wq[:, k], start=(k == 0), stop=(k == NK - 1))
                    fillers(FILL_EARLY.get(k, 0))
                qns = {}
                for mi in range(KOUT):
                    qns[mi] = qk_postproc(pss[mi], mi)
                fillers(FILL)  # cover the PSUM-bank recycle latency

                def proj_block(wt, m):
                    ps = psB.tile([128, GD], F32, tag="ps", bufs=3, name="psb")
                    for k in range(NK):
                        nc.tensor.matmul(
                            ps[:], hs[:, k * S + m * 128: k * S + (m + 1) * 128],
                            wt[:, k], start=(k == 0), stop=(k == NK - 1))
                    return ps

                for m in range(KOUT, NB):
                    qns[m] = qk_postproc(proj_block(wq, m), m)

                def transpose_block(qn, m, dstT):
                    for h in range(HL):
                        pt = psB.tile([128, 128], BF16, tag="pt", bufs=1, name="pt")
                        nc.tensor.transpose(pt[:], qn[:, h], ident[:])
                        nc.vector.tensor_copy(dstT[h][:, m * 128:(m + 1) * 128], pt[:])

                def attn_unit(h, c):
                    nblk = 4 * (c + 1)
                    po = psB.tile([128, 512], F32, tag="po", bufs=2, name="po")
                    ats = []

                    def qk(j):
                        jj = j - 4 * c
                        off = max(jj, 0) * 128  # all-zero cols skipped
                        pa = psB.tile([128, 512], F32, tag="pa", bufs=2, name="pa")
                        nc.tensor.matmul(pa[:, off:512],
                                         kT[h][:, j * 128:(j + 1) * 128],
                                         qT[h][:, c * 512 + off:(c + 1) * 512],
                                         start=True, stop=True)
                        at = atn_pool.tile([128, 512], BF16, tag="at")
                        if jj >= 0:  # diagonal band: apply causal mask
                            nc.vector.tensor_tensor(out=at[:, off:512],
                                                    in0=pa[:, off:512],
                                                    in1=masks[:, jj, off:512],
                                                    op=MUL)
                        else:        # fully below the diagonal
                            nc.scalar.copy(at[:], pa[:])
                        ats.append((at, off))

                    def av(j):
                        at, off = ats[j]
                        nc.tensor.matmul(po[:, off:512],
                                         vn[j][:, h * 128:(h + 1) * 128],
                                         at[:, off:512],
                                         start=(j == 0), stop=(j == nblk - 1))

                    for j in range(nblk):
                        qk(j)
                        if j >= 1:
                            av(j - 1)
                    av(nblk - 1)
                    nc.scalar.copy(aT[h][:, c * 512:(c + 1) * 512], po[:])

                def outproj_block(m):
                    ot = ost_pool.tile([128, E], BF16, tag="ot")
                    for n in range(4):
                        ps = psB.tile([128, 512], F32, tag="ps", bufs=3, name="pso")
                        for kk in range(HL):
                            nc.tensor.matmul(ps[:], aT[kk][:, m * 128:(m + 1) * 128],
                                             wo[:, n, kk],
                                             start=(kk == 0), stop=(kk == HL - 1))
                        dst = ot[:, n * 512:(n + 1) * 512]
                        if n % 2 == 0:
                            nc.vector.tensor_copy(dst, ps[:])
                        else:
                            nc.scalar.copy(dst, ps[:])
                        # per-chunk DMA right after the evict: shortens the
                        # final drain after the last matmul
                        nc.sync.dma_start(
                            out=outd[m * 128:(m + 1) * 128, n * 512:(n + 1) * 512],
                            in_=dst)

                # K projection; interleave lagged Q transposes
                kns = {}
                for m in range(NB):
                    ps = proj_block(wk, m)
                    kns[m] = qk_postproc(ps, m)
                    transpose_block(qns.pop(m), m, qT)

                # V projection; interleave lagged K transposes, then the
                # attention c=0 units (Vector-eviction-heavy) into the
                # PE-rich projection stream
                for m in range(NB):
                    ps = proj_block(wv, m)
                    for h in range(HL):
                        nc.vector.tensor_scalar_mul(vn[m][:, h * 128:(h + 1) * 128],
                                                    ps[:, h * 128:(h + 1) * 128],
                                                    vscale[:, m, h:h + 1])
                    transpose_block(kns.pop(m), m, kT)
                    if m >= 4:
                        attn_unit(m - 4, 0)

                # attention c=1 units interleaved with out-proj blocks
                for i in range(4):
                    attn_unit(i, 1)
                    outproj_block(i)
                for m in range(4, NB):
                    outproj_block(m)

    nc.compile()
    _BUILT = nc
    return nc


def _prep_inputs(hidden_states, w_q, w_k, w_v, w_o, norm_const,
                 attention_mask, position_ids):
    """Host-side shard + table prep. Returns list of 8 in_maps."""
    import ml_dtypes
    BF = ml_dtypes.bfloat16
    hidden_states = np.asarray(hidden_states, dtype=np.float32)
    w_q = np.asarray(w_q, dtype=np.float32)
    w_k = np.asarray(w_k, dtype=np.float32)
    w_v = np.asarray(w_v, dtype=np.float32)
    w_o = np.asarray(w_o, dtype=np.float32)
    norm_const = np.asarray(norm_const, dtype=np.float32).reshape(H)
    attention_mask = np.asarray(attention_mask, dtype=np.float32).reshape(B, S)
    position_ids = np.asarray(position_ids).reshape(B, S).astype(np.int64)

    embed = _sinusoidal(MAXP, ROT)                       # [MAXP, 64]
    sig = 1.0 / (1.0 + np.exp(-norm_const.astype(np.float64)))   # [H]
    mask0 = (attention_mask == 0).astype(np.float32)     # [B, S]
    counts = np.cumsum(mask0, axis=1).astype(np.float32)  # [B, S]
    denom = np.maximum(counts[:, None, :] ** sig[None, :, None], 1.0).astype(np.float32)
    vs_full = mask0[:, None, :] / denom                  # [B, H, S]

    # causal masks for the 4 diagonal-band block offsets
    p = np.arange(128)[:, None]
    f = np.arange(512)[None, :]
    masks = np.stack([(jj * 128 + p <= f) for jj in range(4)]).astype(np.float32)
    masks = np.ascontiguousarray(masks.transpose(1, 0, 2))  # [128, 4, 512]
    ident = np.eye(128, dtype=np.float32).astype(BF)

    def shuffle_k(a):  # [NK*128, F] -> [128, NK, F] (partition-major)
        nk, f = a.shape[0] // 128, a.shape[1]
        return np.ascontiguousarray(
            a.reshape(nk, 128, f).transpose(1, 0, 2).astype(BF))

    in_maps = []
    for b in range(B):
        sincos = embed[position_ids[b]]                  # [S, 64]
        sin, cos = sincos[:, :ROT // 2], sincos[:, ROT // 2:]
        cosR = np.repeat(cos, 2, axis=1)                 # [S, 64]
        sinS = np.empty((S, ROT), dtype=np.float32)
        sinS[:, 0::2] = -sin
        sinS[:, 1::2] = sin
        # [S,64] -> [128 part, NB, 64] -> broadcast over HL heads
        def to4(t):
            t = t.reshape(NB, 128, ROT).transpose(1, 0, 2)
            return np.ascontiguousarray(
                np.broadcast_to(t[:, :, None, :], (128, NB, HL, ROT))).astype(BF)
        cos4 = to4(cosR)
        sin4 = to4(sinS)
        qm = np.ascontiguousarray(mask0[b].reshape(NB, 128).T)  # [128, NB]
        hsp_b = shuffle_k(np.ascontiguousarray(hidden_states[b].T))  # [128,NK,S]
        for g in range(4):
            sl = slice(g * GD, (g + 1) * GD)
            vs = vs_full[b, 4 * g:4 * g + HL, :]                # [HL, S]
            vs = np.ascontiguousarray(
                vs.reshape(HL, NB, 128).transpose(2, 1, 0))     # [128, NB, HL]
            # w_o[:, sl] is [E, GD]; wop[p, n, kk, c] = w_o[n*512+c, kk*128+p]
            wop = np.ascontiguousarray(
                w_o[:, sl].reshape(4, 512, HL, 128).transpose(3, 0, 2, 1)).astype(BF)
            in_maps.append({
                "hsp": hsp_b,
                "wqp": shuffle_k(np.ascontiguousarray(w_q[sl, :].T)),
                "wkp": shuffle_k(np.ascontiguousarray(w_k[sl, :].T)),
                "wvp": shuffle_k(np.ascontiguousarray(w_v[sl, :].T)),
                "wop": wop,
                "cos4": cos4, "sin4": sin4, "masks": masks,
                "vscale": vs, "qmask": qm, "ident": ident,
            })
    # core order: c = b*4 + g
    return in_maps


def run(inputs, trace=False, trace_cores=None):
    from concourse.bass_utils import run_bass_kernel_spmd
    nc = _build()
    in_maps = _prep_inputs(**inputs)
    res = run_bass_kernel_spmd(nc, in_maps, core_ids=list(range(8)),
                               trace=trace, trace_cores=trace_cores)
    out = np.empty((B, S, E), dtype=np.float32)
    for b in range(B):
        acc = np.zeros((S, E), dtype=np.float32)
        for g in range(4):
            acc += np.asarray(res.results[4 * b + g]["out"], dtype=np.float32)
        out[b] = acc
    return out, res


def kernel(**inputs):
    out, _ = run(inputs, trace=False)
    return out


# revision 32
# speedup vs baseline: 1.0385x; 1.0200x over previous
# Trainium2 Bass kernel for GPT-J-style cosine attention (no softmax).
#
# Reference computation (B=2, S=1024, E=2048, H=16, HD=128, ROT=64):
#   q/k/v = hs @ W.T ; partial rotary on first 64 dims of each head;
#   v /= max(count^sigmoid(norm_const), 1); q,k L2-normalized; q,k,v
#   masked by attention_mask==0 rows; attn = tril(q @ k.T) (zeros, no
#   softmax); out = (attn @ v) @ w_o.T.
#
# Sharding: core c = b*4 + g  (b in 0..1 batch, g in 0..3 head-group of
# 4 heads). Each core computes its batch's S x 512 slice of q/k/v, runs
# attention for its 4 heads, and produces a partial [S, E] out-proj
# contribution; the host sums the 4 partials per batch.
#
# All matmul operands are bf16 (halves DMA + faster PE); accumulation
# stays fp32 in PSUM. The schedule keeps the PE continuously busy
# (idle gaps reset the HW clock p-state to 1.2/0.65 GHz):
#  - warmup/filler matmuls on a memset tile cover the DMA fill window
#  - Q projection runs k-outer over 7 PSUM banks so per-k PE work
#    exceeds the DMA arrival pace of the (hs_k, wq_k) tile stream
#  - one PSUM pool for the whole kernel (tags ps/pt/pa/po = 3+1+2+2
#    banks) -- no pool-transition barriers
#  - transposes of q/k lag into the following projection phases
#  - attention c=0 units interleave into the V-projection stream and
#    out-proj blocks into the attention c=1 stream, so PSUM evictions
#    (Vector diag-mask multiplies, Scalar copies) never outpace the PE
#  - all-zero column ranges of diagonal attention blocks are skipped
#    in the qk matmul, the eviction, and the av matmul
#  - out-proj evicts DMA out per 512-column chunk to shorten the drain.
import numpy as np

B, S, E, H, HD, ROT, MAXP = 2, 1024, 2048, 16, 128, 64, 2048
HL = 4            # heads per core
GD = HL * HD      # 512 output dims per core
NB = S // 128     # 8 s-blocks
NK = E // 128     # 16 contraction tiles
EPS = 1e-12
WARM = 10         # warmup matmuls before first projection matmul
KOUT = 7          # m-blocks accumulated k-outer during the DMA fill
FILL = 13         # filler matmuls between k-outer end and m7
FILL_EARLY = {0: 4, 1: 6, 2: 2}  # fillers inside the k-outer loop
VEC_SS = True     # sum-of-squares on Vector (tensor_tensor_reduce) vs Scalar
SCL_QN = True     # qn eviction on Scalar (activation w/ scale) vs Vector


def _sinusoidal(num_pos, dim):
    inv_freq = 1.0 / (10000.0 ** (np.arange(0, dim, 2, dtype=np.float32) / dim))
    sinusoid = np.einsum("i,j->ij", np.arange(num_pos, dtype=np.float32), inv_freq)
    return np.concatenate([np.sin(sinusoid), np.cos(sinusoid)], axis=-1)


_BUILT = None


def _build():
    global _BUILT
    if _BUILT is not None:
        return _BUILT
    import concourse.bacc as bacc
    import concourse.mybir as mybir
    import concourse.bass as bass
    from concourse.tile import TileContext

    F32 = mybir.dt.float32
    BF16 = mybir.dt.bfloat16
    MUL = mybir.AluOpType.mult
    SQUARE = mybir.ActivationFunctionType.Square

    nc = bacc.Bacc(None, target_bir_lowering=False)

    hsd = nc.dram_tensor("hsp", [128, NK, S], BF16, kind="ExternalInput")
    wqd = nc.dram_tensor("wqp", [128, NK, GD], BF16, kind="ExternalInput")
    wkd = nc.dram_tensor("wkp", [128, NK, GD], BF16, kind="ExternalInput")
    wvd = nc.dram_tensor("wvp", [128, NK, GD], BF16, kind="ExternalInput")
    wod = nc.dram_tensor("wop", [128, 4, HL, 512], BF16, kind="ExternalInput")
    cos4d = nc.dram_tensor("cos4", [128, NB, HL, ROT], BF16, kind="ExternalInput")
    sin4d = nc.dram_tensor("sin4", [128, NB, HL, ROT], BF16, kind="ExternalInput")
    masksd = nc.dram_tensor("masks", [128, 4, 512], F32, kind="ExternalInput")
    vscaled = nc.dram_tensor("vscale", [128, NB, HL], F32, kind="ExternalInput")
    qmaskd = nc.dram_tensor("qmask", [128, NB], F32, kind="ExternalInput")
    identd = nc.dram_tensor("ident", [128, 128], BF16, kind="ExternalInput")
    outd = nc.dram_tensor("out", [S, E], BF16, kind="ExternalOutput")

    with TileContext(nc) as tc:
        from contextlib import ExitStack
        ctx = ExitStack()
        with ctx:
            const = ctx.enter_context(tc.tile_pool(name="const", bufs=1))
            data = ctx.enter_context(tc.tile_pool(name="data", bufs=1))
            scr = ctx.enter_context(tc.tile_pool(name="scr", bufs=4))
            rot_pool = ctx.enter_context(tc.tile_pool(name="rot", bufs=2))

            junk = const.tile([128, 512], BF16)
            cos4 = const.tile([128, NB, HL, ROT], BF16)
            sin4 = const.tile([128, NB, HL, ROT], BF16)
            masks = const.tile([128, 4, 512], F32)
            vscale = const.tile([128, NB, HL], F32)
            qmask = const.tile([128, NB], F32)
            ident = const.tile([128, 128], BF16)

            # junk tile for warmup matmuls: locally initialized, no DMA wait
            nc.gpsimd.memset(junk[:], 0.125)
            # small consts on the scalar queue (land within ~10us)

            # persistent data tiles
            hs = data.tile([128, NK * S], BF16)
            wq = data.tile([128, NK, GD], BF16)
            wk = data.tile([128, NK, GD], BF16)
            wv = data.tile([128, NK, GD], BF16)
            qT = [data.tile([128, S], BF16, name=f"qT{h}") for h in range(HL)]
            kT = [data.tile([128, S], BF16, name=f"kT{h}") for h in range(HL)]
            vn = [data.tile([128, GD], BF16, name=f"vn{m}") for m in range(NB)]
            aT = [data.tile([128, S], BF16, name=f"aT{h}") for h in range(HL)]
            wo = data.tile([128, 4, HL, 512], BF16)

            # sync-queue DMA stream: fine-grained (hs_k, wq_k) pairs first
            # (paces the Q projection), then wk/wv groups, then the
            # late-needed tables. The DGE processes these in order, so the
            # early stream never competes for HBM with the late loads.
            nc.scalar.dma_start(out=qmask[:], in_=qmaskd[:])
            nc.scalar.dma_start(out=ident[:], in_=identd[:])
            for k in range(NK):
                nc.sync.dma_start(
                    out=hs[:, k * S:(k + 1) * S],
                    in_=bass.AP(hsd, k * S, [[NK * S, 128], [1, S]]))
                nc.sync.dma_start(
                    out=wq[:, k],
                    in_=bass.AP(wqd, k * GD, [[NK * GD, 128], [1, GD]]))
            nc.sync.dma_start(out=cos4[:], in_=cos4d[:])
            nc.sync.dma_start(out=sin4[:], in_=sin4d[:])
            for g in range(4):
                nc.sync.dma_start(
                    out=wk[:, 4 * g:4 * (g + 1)],
                    in_=bass.AP(wkd, 4 * g * GD, [[NK * GD, 128], [1, 4 * GD]]))
            for g in range(4):
                nc.sync.dma_start(
                    out=wv[:, 4 * g:4 * (g + 1)],
                    in_=bass.AP(wvd, 4 * g * GD, [[NK * GD, 128], [1, 4 * GD]]))
            nc.sync.dma_start(out=vscale[:], in_=vscaled[:])
            nc.sync.dma_start(out=masks[:], in_=masksd[:])
            nc.sync.dma_start(out=wo[:], in_=wod[:])

            ADD = mybir.AluOpType.add

            def qk_postproc(ps, m):
                # per-head L2 norms straight from PSUM (rotary is
                # norm-preserving so norms can be taken pre-rotary).
                # Split across Vector (sum-of-squares) and Scalar (scaled
                # eviction) so PSUM banks release at ~1.8us/block pace.
                ss = scr.tile([128, HL], F32, tag="ss")
                if VEC_SS:
                    # one Scalar square over all 4 heads + one Vector
                    # segmented reduce: cheapest PSUM-release path
                    sqs = scr.tile([128, HL, 128], F32, tag="sqs", bufs=2)
                    nc.scalar.activation(out=sqs[:], in_=ps[:], func=SQUARE)
                    nc.vector.tensor_reduce(out=ss[:], in_=sqs[:],
                                            axis=mybir.AxisListType.X, op=ADD)
                else:
                    sqs = scr.tile([128, 128], F32, tag="sqs", bufs=2)
                    for h in range(HL):
                        nc.scalar.activation(out=sqs[:],
                                             in_=ps[:, h * 128:(h + 1) * 128],
                                             func=SQUARE,
                                             accum_out=ss[:, h:h + 1])
                nrm = scr.tile([128, HL], F32, tag="nrm")
                nc.scalar.sqrt(nrm[:], ss[:])
                nc.vector.tensor_scalar_max(nrm[:], nrm[:], EPS)
                rr = scr.tile([128, HL], F32, tag="rr")
                nc.vector.reciprocal(rr[:], nrm[:])
                nc.vector.tensor_scalar_mul(rr[:], rr[:], qmask[:, m:m + 1])
                # PSUM -> SBUF bf16 with the per-row scale folded in
                qn = rot_pool.tile([128, HL, 128], BF16, tag="qn", bufs=10)
                for h in range(HL):
                    nc.scalar.mul(qn[:, h], ps[:, h * 128:(h + 1) * 128],
                                  rr[:, h:h + 1])
                # GPT-J interleaved rotary on first ROT dims of each head
                qrot = rot_pool.tile([128, HL, ROT], BF16, tag="qrot", bufs=2)
                tmp2 = rot_pool.tile([128, HL, ROT], BF16, tag="tmp2", bufs=2)
                nc.gpsimd.tensor_tensor(out=qrot[:, :, 0:ROT:2], in0=qn[:, :, 1:ROT:2],
                                        in1=sin4[:, m, :, 0:ROT:2], op=MUL)
                nc.gpsimd.tensor_tensor(out=qrot[:, :, 1:ROT:2], in0=qn[:, :, 0:ROT:2],
                                        in1=sin4[:, m, :, 1:ROT:2], op=MUL)
                nc.gpsimd.tensor_tensor(out=tmp2[:], in0=qn[:, :, 0:ROT],
                                        in1=cos4[:, m], op=MUL)
                nc.gpsimd.tensor_add(out=qn[:, :, 0:ROT], in0=qrot[:], in1=tmp2[:])
                return qn

            # One PSUM pool for the whole kernel (no pool-transition
            # barriers): ps(3) + pt(1) + pa(2) + po(2) = 8 banks.
            with tc.tile_pool(name="psB", bufs=1, space="PSUM") as psB, \
                 tc.tile_pool(name="atn", bufs=8) as atn_pool, \
                 tc.tile_pool(name="ost", bufs=3) as ost_pool:

                def fillers(n):
                    for _ in range(n):
                        wp = psB.tile([128, 512], F32, tag="pt", bufs=1, name="wp")
                        nc.tensor.matmul(wp[:], junk[:, 0:128], junk[:],
                                         start=True, stop=True)

                fillers(WARM)

                # ---- Q projection: k-outer over 7 m-blocks (borrowing the
                # attention pa/po banks) so the PE keeps pace with the
                # arriving (hs_k, wq_k) DMA stream.
                ptags = [("ps", 3)] * 3 + [("pa", 2)] * 2 + [("po", 2)] * 2
                pss = [psB.tile([128, GD], F32, tag=t, bufs=b, name=f"pss{i}")
                       for i, (t, b) in enumerate(ptags[:KOUT])]
                for k in range(NK):
                    for mi in range(KOUT):
                        nc.tensor.matmul(
                            pss[mi][:], hs[:, k * S + mi * 128: k * S + (mi + 1) * 128],
                            wq[:, k], start=(k == 0), stop=(k == NK - 1))
                    fillers(FILL_EARLY.get(k, 0))
                qns = {}
                for mi in range(KOUT):
                    qns[mi] = qk_postproc(pss[mi], mi)
                fillers(FILL)  # cover the PSUM-bank recycle latency

                def proj_block(wt, m):
                    ps = psB.tile([128, GD], F32, tag="ps", bufs=3, name="psb")
                    for k in range(NK):
                        nc.tensor.matmul(
                            ps[:], hs[:, k * S + m * 128: k * S + (m + 1) * 128],
                            wt[:, k], start=(k == 0), stop=(k == NK - 1))
                    return ps

                for m in range(KOUT, NB):
                    qns[m] = qk_postproc(proj_block(wq, m), m)

                def transpose_block(qn, m, dstT):
                    for h in range(HL):
                        pt = psB.tile([128, 128], BF16, tag="pt", bufs=1, name="pt")
                        nc.tensor.transpose(pt[:], qn[:, h], ident[:])
                        nc.vector.tensor_copy(dstT[h][:, m * 128:(m + 1) * 128], pt[:])

                def attn_unit(h, c):
                    nblk = 4 * (c + 1)
                    po = psB.tile([128, 512], F32, tag="po", bufs=2, name="po")
                    ats = []

                    def qk(j):
                        jj = j - 4 * c
                        off = max(jj, 0) * 128  # all-zero cols skipped
                        pa = psB.tile([128, 512], F32, tag="pa", bufs=2, name="pa")
                        nc.tensor.matmul(pa[:, off:512],
                                         kT[h][:, j * 128:(j + 1) * 128],
                                         qT[h][:, c * 512 + off:(c + 1) * 512],
                                         start=True, stop=True)
                        at = atn_pool.tile([128, 512], BF16, tag="at")
                        if jj >= 0:  # diagonal band: apply causal mask
                            nc.vector.tensor_tensor(out=at[:, off:512],
                                                    in0=pa[:, off:512],
                                                    in1=masks[:, jj, off:512],
                                                    op=MUL)
                        else:        # fully below the diagonal
                            nc.scalar.copy(at[:], pa[:])
                        ats.append((at, off))

                    def av(j):
                        at, off = ats[j]
                        nc.tensor.matmul(po[:, off:512],
                                         vn[j][:, h * 128:(h + 1) * 128],
                                         at[:, off:512],
                                         start=(j == 0), stop=(j == nblk - 1))

                    for j in range(nblk):
                        qk(j)
                        if j >= 1:
                            av(j - 1)
                    av(nblk - 1)
                    nc.scalar.copy(aT[h][:, c * 512:(c + 1) * 512], po[:])

                def outproj_block(m):
                    ot = ost_pool.tile([128, E], BF16, tag="ot")
                    for n in range(4):
                        ps = psB.tile([128, 512], F32, tag="ps", bufs=3, name="pso")
                        for kk in range(HL):
                            nc.tensor.matmul(ps[:], aT[kk][:, m * 128:(m + 1) * 128],
                                             wo[:, n, kk],
                                             start=(kk == 0), stop=(kk == HL - 1))
                        dst = ot[:, n * 512:(n + 1) * 512]
                        if n % 2 == 0:
                            nc.vector.tensor_copy(dst, ps[:])
                        else:
                            nc.scalar.copy(dst, ps[:])
                        # per-chunk DMA right after the evict: shortens the
                        # final drain after the last matmul
                        nc.sync.dma_start(
                            out=outd[m * 128:(m + 1) * 128, n * 512:(n + 1) * 512],
                            in_=dst)

                # K projection; interleave lagged Q transposes
                kns = {}
                for m in range(NB):
                    ps = proj_block(wk, m)
                    kns[m] = qk_postproc(ps, m)
                    transpose_block(qns.pop(m), m, qT)

                # V projection; interleave lagged K transposes, then the
                # attention c=0 units (Vector-eviction-heavy) into the
                # PE-rich projection stream
                for m in range(NB):
                    ps = proj_block(wv, m)
                    for h in range(HL):
                        nc.vector.tensor_scalar_mul(vn[m][:, h * 128:(h + 1) * 128],
                                                    ps[:, h * 128:(h + 1) * 128],
                                                    vscale[:, m, h:h + 1])
                    transpose_block(kns.pop(m), m, kT)
                    if m >= 4:
                        attn_unit(m - 4, 0)

                # attention c=1 units interleaved with out-proj blocks
                for i in range(4):
                    attn_unit(i, 1)
                    outproj_block(i)
                for m in range(4, NB):
                    outproj_block(m)

    nc.compile()
    _BUILT = nc
    return nc


def _prep_inputs(hidden_states, w_q, w_k, w_v, w_o, norm_const,
                 attention_mask, position_ids):
    """Host-side shard + table prep. Returns list of 8 in_maps."""
    import ml_dtypes
    BF = ml_dtypes.bfloat16
    hidden_states = np.asarray(hidden_states, dtype=np.float32)
    w_q = np.asarray(w_q, dtype=np.float32)
    w_k = np.asarray(w_k, dtype=np.float32)
    w_v = np.asarray(w_v, dtype=np.float32)
    w_o = np.asarray(w_o, dtype=np.float32)
    norm_const = np.asarray(norm_const, dtype=np.float32).reshape(H)
    attention_mask = np.asarray(attention_mask, dtype=np.float32).reshape(B, S)
    position_ids = np.asarray(position_ids).reshape(B, S).astype(np.int64)

    embed = _sinusoidal(MAXP, ROT)                       # [MAXP, 64]
    sig = 1.0 / (1.0 + np.exp(-norm_const.astype(np.float64)))   # [H]
    mask0 = (attention_mask == 0).astype(np.float32)     # [B, S]
    counts = np.cumsum(mask0, axis=1).astype(np.float32)  # [B, S]
    denom = np.maximum(counts[:, None, :] ** sig[None, :, None], 1.0).astype(np.float32)
    vs_full = mask0[:, None, :] / denom                  # [B, H, S]

    # causal masks for the 4 diagonal-band block offsets
    p = np.arange(128)[:, None]
    f = np.arange(512)[None, :]
    masks = np.stack([(jj * 128 + p <= f) for jj in range(4)]).astype(np.float32)
    masks = np.ascontiguousarray(masks.transpose(1, 0, 2))  # [128, 4, 512]
    ident = np.eye(128, dtype=np.float32).astype(BF)

    def shuffle_k(a):  # [NK*128, F] -> [128, NK, F] (partition-major)
        nk, f = a.shape[0] // 128, a.shape[1]
        return np.ascontiguousarray(
            a.reshape(nk, 128, f).transpose(1, 0, 2).astype(BF))

    in_maps = []
    for b in range(B):
        sincos = embed[position_ids[b]]                  # [S, 64]
        sin, cos = sincos[:, :ROT // 2], sincos[:, ROT // 2:]
        cosR = np.repeat(cos, 2, axis=1)                 # [S, 64]
        sinS = np.empty((S, ROT), dtype=np.float32)
        sinS[:, 0::2] = -sin
        sinS[:, 1::2] = sin
        # [S,64] -> [128 part, NB, 64] -> broadcast over HL heads
        def to4(t):
            t = t.reshape(NB, 128, ROT).transpose(1, 0, 2)
            return np.ascontiguousarray(
                np.broadcast_to(t[:, :, None, :], (128, NB, HL, ROT))).astype(BF)
        cos4 = to4(cosR)
        sin4 = to4(sinS)
        qm = np.ascontiguousarray(mask0[b].reshape(NB, 128).T)  # [128, NB]
        hsp_b = shuffle_k(np.ascontiguousarray(hidden_states[b].T))  # [128,NK,S]
        for g in range(4):
            sl = slice(g * GD, (g + 1) * GD)
            vs = vs_full[b, 4 * g:4 * g + HL, :]                # [HL, S]
            vs = np.ascontiguousarray(
                vs.reshape(HL, NB, 128).transpose(2, 1, 0))     # [128, NB, HL]
            # w_o[:, sl] is [E, GD]; wop[p, n, kk, c] = w_o[n*512+c, kk*128+p]
            wop = np.ascontiguousarray(
                w_o[:, sl].reshape(4, 512, HL, 128).transpose(3, 0, 2, 1)).astype(BF)
            in_maps.append({
                "hsp": hsp_b,
                "wqp": shuffle_k(np.ascontiguousarray(w_q[sl, :].T)),
                "wkp": shuffle_k(np.ascontiguousarray(w_k[sl, :].T)),
                "wvp": shuffle_k(np.ascontiguousarray(w_v[sl, :].T)),
                "wop": wop,
                "cos4": cos4, "sin4": sin4, "masks": masks,
                "vscale": vs, "qmask": qm, "ident": ident,
            })
    # core order: c = b*4 + g
    return in_maps


def run(inputs, trace=False, trace_cores=None):
    from concourse.bass_utils import run_bass_kernel_spmd
    nc = _build()
    in_maps = _prep_inputs(**inputs)
    res = run_bass_kernel_spmd(nc, in_maps, core_ids=list(range(8)),
                               trace=trace, trace_cores=trace_cores)
    out = np.empty((B, S, E), dtype=np.float32)
    for b in range(B):
        acc = np.zeros((S, E), dtype=np.float32)
        for g in range(4):
            acc += np.asarray(res.results[4 * b + g]["out"], dtype=np.float32)
        out[b] = acc
    return out, res


def kernel(**inputs):
    out, _ = run(inputs, trace=False)
    return out
